# revision 10
# baseline (speedup 1.0000x reference)
"""Triangular matmul C = triu(triu(A) @ triu(B)) on 8 TRN2 NeuronCores.

Structure: the (I, K, J) block-tetrahedron {I <= K <= J} (128x128 blocks,
N=4096 -> 32 blocks/side) is sharded by output row-block I across the 8
cores with a work-balanced assignment.  Each core runs its own statically
addressed program inside a `tc.If(partition_id == c)` block.

v2 design (vs the bf16x3 baseline at ~270us):
- Single bf16 pass (harness gate is 2e-2; measured host-sim rel err ~2e-3).
  3x less PE work, 2x less DMA than bf16x3.
- B strips are SBUF-resident: strip K = B[K*128:(K+1)*128, K*128:] is DMA'd
  ONCE per core (132KB/partition worst case) instead of once per (I,K).
- A^T blocks for all owned rows are loaded up-front (<=20KB/partition).
- Per row I: K-major sweep, full output row in up to 8 PSUM banks; bank c
  is evicted EARLY (as soon as its last contributing K = 4c+3 completes),
  overlapping eviction with the remaining K sweep.
- C is stored as bf16 (host converts to fp32; adds <=2e-3 elementwise).

The kernel takes FULL (unsharded) inputs and returns the FULL output.
"""

import numpy as np

N = 4096
BLK = 128
NB = N // BLK  # 32
N_CORES = 8
PHASE = 512  # PSUM bank width (fp32)
NBANK = 8

# Work-balanced assignment of row-blocks I to cores (work(I) = T(32-I),
# T(m)=m(m+1)/2; bins balanced to 743..752 of 5984/8=748).
BINS = [
    [0, 14, 23],
    [1, 15, 21, 25, 29],
    [2, 13, 20, 28],
    [4, 12, 16],
    [3, 10, 22],
    [6, 9, 17, 30],
    [5, 11, 19, 24, 27, 31],
    [7, 8, 18, 26],
]
MAXB = max(len(b) for b in BINS)  # output row-slots per core
# A-pack slot layout: per core, the A^T strips (one 128x128 block per slot)
# for each owned I, K = I..31, concatenated.
ABASE = [
    {I: int(np.cumsum([0] + [NB - J for J in sorted(b)])[i]) for i, I in enumerate(sorted(b))}
    for b in BINS
]
NSLOT = 80  # >= max per-core total blocks (75)
ACHUNK = 8  # A-load DMA granularity in k-blocks


def _emit_loads(nc, tc, pools, dram_io):
    """Unconditional (pid-independent) HBM->SBUF loads, issued before the
    Switch dispatch so they stream during its latency.  B strips and A slots
    are issued in DESCENDING K / slot order to match the descending-I row
    order inside the arms (each core's first row needs the strips that
    arrive earliest)."""
    import concourse.mybir as mybir

    bf16 = mybir.dt.bfloat16
    apool, bpool, cpool, psum_pool = pools
    apack, bh = dram_io["apack"], dram_io["bh"]

    a_t = apool.tile([BLK, NSLOT, BLK], bf16, name="a_all", tag="a", bufs=1)
    # A slots descending (high slots hold high-I rows of the LAST bins; the
    # per-core need order varies, but A is small and lands quickly anyway).
    for j1 in range(NSLOT, 0, -ACHUNK):
        j0 = max(j1 - ACHUNK, 0)
        nc.gpsimd.dma_start(a_t[:, j0:j1, :], apack[:, j0:j1, :])

    beng = [nc.sync, nc.scalar]
    bsb = {}
    for i, K in enumerate(range(NB - 1, -1, -1)):
        W = N - K * BLK
        t = bpool.tile([BLK, W], bf16, name=f"bs_{K}", tag=f"bs{K}", bufs=1)
        beng[i % 2].dma_start(t[:], bh[K * BLK : (K + 1) * BLK, K * BLK : N])
        bsb[K] = t
    return a_t, bsb


def _emit_core(nc, tc, pools, dram_io, core, a_all, bsb):
    import concourse.mybir as mybir

    f32 = mybir.dt.float32
    bf16 = mybir.dt.bfloat16
    apool, bpool, cpool, psum_pool = pools
    cpart = dram_io["cpart"]

    bin_is = sorted(BINS[core])
    slot = {I: s for s, I in enumerate(bin_is)}

    # --- compute, rows descending I (first row needs earliest strips) ---
    cpeng = [
        lambda o, i: nc.vector.tensor_copy(o, i),
        lambda o, i: nc.scalar.copy(o, i),
    ]  # PSUM -> SBUF eviction copies
    steng = [nc.gpsimd, nc.sync]  # SBUF -> HBM stores
    ev = 0
    for I in sorted(bin_is, reverse=True):
        base = ABASE[core][I]
        c0 = I // 4
        ps = {
            c: psum_pool.tile([BLK, PHASE], f32, name=f"ps_{I}_{c}", tag=f"ps{c}")
            for c in range(c0, NBANK)
        }
        for K in range(I, NB):
            a_w = a_all[:, base + K - I, :]
            bt = bsb[K]
            for c in range(K // 4, NBANK):
                cstart = max(K * BLK, PHASE * c)
                cwidth = PHASE * (c + 1) - cstart
                boff = cstart - K * BLK
                o = ps[c][:, cstart - PHASE * c : PHASE]
                nc.tensor.matmul(
                    o, a_w, bt[:, boff : boff + cwidth],
                    start=(K == I), stop=(K == min(4 * c + 3, NB - 1)),
                )
            # early-evict banks whose last contributing K just ran
            for c in range(c0, NBANK):
                if min(4 * c + 3, NB - 1) == K:
                    coff0 = max(I * BLK - PHASE * c, 0)
                    w = PHASE - coff0
                    ct = cpool.tile([BLK, PHASE], bf16, name=f"c_{I}_{c}", tag="cst")
                    cpeng[ev % 2](ct[:, :w], ps[c][:, coff0:PHASE])
                    r0 = slot[I] * BLK
                    steng[ev % 2].dma_start(
                        cpart[r0 : r0 + BLK, PHASE * c + coff0 : PHASE * (c + 1)],
                        ct[:, :w],
                    )
                    ev += 1


def _build():
    import concourse.mybir as mybir
    import concourse.tile as tile
    from concourse import bacc

    nc = bacc.Bacc(None, target_bir_lowering=False, debug=False)
    f32 = mybir.dt.float32
    bf16 = mybir.dt.bfloat16
    with tile.TileContext(nc) as tc:
        with (
            tc.tile_pool(name="dram", bufs=1, space="DRAM") as dram,
            tc.tile_pool(name="apool", bufs=1) as apool,
            tc.tile_pool(name="bpool", bufs=1) as bpool,
            tc.tile_pool(name="cpool", bufs=4) as cpool,
            tc.tile_pool(name="psum", bufs=1, space="PSUM") as psum_pool,
        ):
            dram_io = {
                "apack": dram.tile(
                    [BLK, NSLOT, BLK], bf16, kind="ExternalInput",
                    name="apack", uniquify=False,
                ),
                "bh": dram.tile(
                    [N, N], bf16, kind="ExternalInput", name="bh", uniquify=False,
                ),
                "cpart": dram.tile(
                    [MAXB * BLK, N], bf16, kind="ExternalOutput",
                    name="cpart", uniquify=False,
                ),
            }
            pools = (apool, bpool, cpool, psum_pool)
            a_all, bsb = _emit_loads(nc, tc, pools, dram_io)
            pid = nc.partition_id()
            for c in tc.Switch(pid, N_CORES):
                _emit_core(nc, tc, pools, dram_io, c, a_all, bsb)
    nc.compile()
    return nc


_cached_nc = None


def _get_nc():
    global _cached_nc
    if _cached_nc is None:
        _cached_nc = _build()
    return _cached_nc


def _host_pack(A, B):
    """Build per-core apack tensors (A^T blocks, bf16) and bh (B, bf16)."""
    import ml_dtypes

    bf16 = ml_dtypes.bfloat16
    ath = np.ascontiguousarray(A.T).astype(bf16)
    bh = np.ascontiguousarray(B.astype(bf16))

    apacks = []
    for c in range(N_CORES):
        ap = np.zeros((BLK, NSLOT, BLK), dtype=bf16)
        for I in BINS[c]:
            base = ABASE[c][I]
            for j, K in enumerate(range(I, NB)):
                ap[:, base + j, :] = ath[
                    K * BLK : (K + 1) * BLK, I * BLK : (I + 1) * BLK
                ]
        apacks.append(ap)
    return apacks, bh


LAST = None  # last BassKernelResults (for test harness introspection)


def kernel(A, B):
    global LAST
    import os

    from concourse.bass_utils import run_bass_kernel_spmd

    A = np.asarray(A, dtype=np.float32)
    B = np.asarray(B, dtype=np.float32)
    nc = _get_nc()
    apacks, bh = _host_pack(A, B)
    in_maps = [{"apack": apacks[c], "bh": bh} for c in range(N_CORES)]
    tkw = {}
    if os.environ.get("KTRACE"):
        tkw["trace"] = True
        tkw["tmpdir"] = os.environ.get("KTRACE_DIR") or None
        tc_env = os.environ.get("KTRACE_CORES")
        if tc_env:
            tkw["trace_cores"] = [int(x) for x in tc_env.split(",")]
    res = run_bass_kernel_spmd(nc, in_maps, core_ids=list(range(N_CORES)), **tkw)
    LAST = res

    C = np.zeros((N, N), dtype=np.float32)
    for c in range(N_CORES):
        cp = res.results[c]["cpart"]
        for s, I in enumerate(sorted(BINS[c])):
            C[I * BLK : (I + 1) * BLK, I * BLK :] = cp[
                s * BLK : (s + 1) * BLK, I * BLK :
            ].astype(np.float32)
    return C


# revision 12
# speedup vs baseline: 1.2013x; 1.2013x over previous
"""Triangular matmul C = triu(triu(A) @ triu(B)) on 8 TRN2 NeuronCores.

Structure: the (I, K, J) block-tetrahedron {I <= K <= J} (128x128 blocks,
N=4096 -> 32 blocks/side) is sharded by output row-block I across the 8
cores with a work-balanced assignment.  Each core runs its own statically
addressed program inside a `tc.If(partition_id == c)` block.

v2 design (vs the bf16x3 baseline at ~270us):
- Single bf16 pass (harness gate is 2e-2; measured host-sim rel err ~2e-3).
  3x less PE work, 2x less DMA than bf16x3.
- B strips are SBUF-resident: strip K = B[K*128:(K+1)*128, K*128:] is DMA'd
  ONCE per core (132KB/partition worst case) instead of once per (I,K).
- A^T blocks for all owned rows are loaded up-front (<=20KB/partition).
- Per row I: K-major sweep, full output row in up to 8 PSUM banks; bank c
  is evicted EARLY (as soon as its last contributing K = 4c+3 completes),
  overlapping eviction with the remaining K sweep.
- C is stored as bf16 (host converts to fp32; adds <=2e-3 elementwise).

The kernel takes FULL (unsharded) inputs and returns the FULL output.
"""

import numpy as np

N = 4096
BLK = 128
NB = N // BLK  # 32
N_CORES = 8
PHASE = 512  # PSUM bank width (fp32)
NBANK = 8

# Work-balanced assignment of row-blocks I to cores (work(I) = T(32-I),
# T(m)=m(m+1)/2; bins balanced to 743..752 of 5984/8=748).
BINS = [
    [0, 14, 23],
    [1, 15, 21, 25, 29],
    [2, 13, 20, 28],
    [4, 12, 16],
    [3, 10, 22],
    [6, 9, 17, 30],
    [5, 11, 19, 24, 27, 31],
    [7, 8, 18, 26],
]
MAXB = max(len(b) for b in BINS)  # output row-slots per core
# A-pack slot layout: per core, the A^T strips (one 128x128 block per slot)
# for each owned I, K = I..31, concatenated.
ABASE = [
    {I: int(np.cumsum([0] + [NB - J for J in sorted(b)])[i]) for i, I in enumerate(sorted(b))}
    for b in BINS
]
NSLOT = 80  # >= max per-core total blocks (75)
ACHUNK = 8  # A-load DMA granularity in k-blocks


def _emit_loads(nc, tc, pools, dram_io):
    """Unconditional (pid-independent) HBM->SBUF loads issued before the
    Switch dispatch: all A slots (apack content is per-core via in_maps),
    plus tiny warmup DMAs on sync/scalar so their ~4.6us first-DMA cost is
    paid during the preamble rather than in front of the first B strip."""
    import concourse.mybir as mybir

    bf16 = mybir.dt.bfloat16
    apool, bpool, cpool, psum_pool = pools
    apack, bh = dram_io["apack"], dram_io["bh"]

    wt = apool.tile([BLK, 8], bf16, name="warm", tag="warm", bufs=1)
    nc.sync.dma_start(wt[:, 0:4], bh[0:BLK, 0:4])
    nc.scalar.dma_start(wt[:, 4:8], bh[0:BLK, 4:8])

    a_t = apool.tile([BLK, NSLOT, BLK], bf16, name="a_all", tag="a", bufs=1)
    for j0 in range(0, NSLOT, ACHUNK):
        j1 = min(j0 + ACHUNK, NSLOT)
        nc.gpsimd.dma_start(a_t[:, j0:j1, :], apack[:, j0:j1, :])
    return a_t


def _emit_core(nc, tc, pools, dram_io, core, a_all):
    import concourse.mybir as mybir

    f32 = mybir.dt.float32
    bf16 = mybir.dt.bfloat16
    apool, bpool, cpool, psum_pool = pools
    bh, cpart = dram_io["bh"], dram_io["cpart"]

    bin_is = sorted(BINS[core])
    Imin = bin_is[0]
    slot = {I: s for s, I in enumerate(bin_is)}

    # --- resident B strips (tailored: only K >= Imin), ascending K ---
    beng = [nc.sync, nc.scalar]
    bsb = {}
    for i, K in enumerate(range(Imin, NB)):
        W = N - K * BLK
        t = bpool.tile([BLK, W], bf16, name=f"bs_{K}", tag=f"bs{K}", bufs=1)
        beng[i % 2].dma_start(t[:], bh[K * BLK : (K + 1) * BLK, K * BLK : N])
        bsb[K] = t

    # --- compute, rows ascending I (paced behind the strip stream) ---
    cpeng = [
        lambda o, i: nc.vector.tensor_copy(o, i),
        lambda o, i: nc.scalar.copy(o, i),
    ]  # PSUM -> SBUF eviction copies
    steng = [nc.gpsimd, nc.sync]  # SBUF -> HBM stores
    ev = 0
    for I in bin_is:
        base = ABASE[core][I]
        c0 = I // 4
        ps = {
            c: psum_pool.tile([BLK, PHASE], f32, name=f"ps_{I}_{c}", tag=f"ps{c}")
            for c in range(c0, NBANK)
        }
        for K in range(I, NB):
            a_w = a_all[:, base + K - I, :]
            bt = bsb[K]
            for c in range(K // 4, NBANK):
                cstart = max(K * BLK, PHASE * c)
                cwidth = PHASE * (c + 1) - cstart
                boff = cstart - K * BLK
                o = ps[c][:, cstart - PHASE * c : PHASE]
                nc.tensor.matmul(
                    o, a_w, bt[:, boff : boff + cwidth],
                    start=(K == I), stop=(K == min(4 * c + 3, NB - 1)),
                )
            # early-evict banks whose last contributing K just ran
            for c in range(c0, NBANK):
                if min(4 * c + 3, NB - 1) == K:
                    coff0 = max(I * BLK - PHASE * c, 0)
                    w = PHASE - coff0
                    ct = cpool.tile([BLK, PHASE], bf16, name=f"c_{I}_{c}", tag="cst")
                    cpeng[ev % 2](ct[:, :w], ps[c][:, coff0:PHASE])
                    r0 = slot[I] * BLK
                    steng[ev % 2].dma_start(
                        cpart[r0 : r0 + BLK, PHASE * c + coff0 : PHASE * (c + 1)],
                        ct[:, :w],
                    )
                    ev += 1


def _build():
    import concourse.mybir as mybir
    import concourse.tile as tile
    from concourse import bacc

    nc = bacc.Bacc(None, target_bir_lowering=False, debug=False)
    f32 = mybir.dt.float32
    bf16 = mybir.dt.bfloat16
    with tile.TileContext(nc) as tc:
        with (
            tc.tile_pool(name="dram", bufs=1, space="DRAM") as dram,
            tc.tile_pool(name="apool", bufs=1) as apool,
            tc.tile_pool(name="bpool", bufs=1) as bpool,
            tc.tile_pool(name="cpool", bufs=4) as cpool,
            tc.tile_pool(name="psum", bufs=1, space="PSUM") as psum_pool,
        ):
            dram_io = {
                "apack": dram.tile(
                    [BLK, NSLOT, BLK], bf16, kind="ExternalInput",
                    name="apack", uniquify=False,
                ),
                "bh": dram.tile(
                    [N, N], bf16, kind="ExternalInput", name="bh", uniquify=False,
                ),
                "cpart": dram.tile(
                    [MAXB * BLK, N], bf16, kind="ExternalOutput",
                    name="cpart", uniquify=False,
                ),
            }
            pools = (apool, bpool, cpool, psum_pool)
            pid = nc.partition_id()
            hint = tc.switch_hint(
                {e: pid for e in mybir.ALL_ENGINES}, N_CORES, label="coresw"
            )
            a_all = _emit_loads(nc, tc, pools, dram_io)
            for c in tc.Switch(pid, N_CORES, hint=hint):
                _emit_core(nc, tc, pools, dram_io, c, a_all)
    nc.compile()
    return nc


_cached_nc = None


def _get_nc():
    global _cached_nc
    if _cached_nc is None:
        _cached_nc = _build()
    return _cached_nc


def _host_pack(A, B):
    """Build per-core apack tensors (A^T blocks, bf16) and bh (B, bf16)."""
    import ml_dtypes

    bf16 = ml_dtypes.bfloat16
    ath = np.ascontiguousarray(A.T).astype(bf16)
    bh = np.ascontiguousarray(B.astype(bf16))

    apacks = []
    for c in range(N_CORES):
        ap = np.zeros((BLK, NSLOT, BLK), dtype=bf16)
        for I in BINS[c]:
            base = ABASE[c][I]
            for j, K in enumerate(range(I, NB)):
                ap[:, base + j, :] = ath[
                    K * BLK : (K + 1) * BLK, I * BLK : (I + 1) * BLK
                ]
        apacks.append(ap)
    return apacks, bh


LAST = None  # last BassKernelResults (for test harness introspection)


def kernel(A, B):
    global LAST
    import os

    from concourse.bass_utils import run_bass_kernel_spmd

    A = np.asarray(A, dtype=np.float32)
    B = np.asarray(B, dtype=np.float32)
    nc = _get_nc()
    apacks, bh = _host_pack(A, B)
    in_maps = [{"apack": apacks[c], "bh": bh} for c in range(N_CORES)]
    tkw = {}
    if os.environ.get("KTRACE"):
        tkw["trace"] = True
        tkw["tmpdir"] = os.environ.get("KTRACE_DIR") or None
        tc_env = os.environ.get("KTRACE_CORES")
        if tc_env:
            tkw["trace_cores"] = [int(x) for x in tc_env.split(",")]
    res = run_bass_kernel_spmd(nc, in_maps, core_ids=list(range(N_CORES)), **tkw)
    LAST = res

    C = np.zeros((N, N), dtype=np.float32)
    for c in range(N_CORES):
        cp = res.results[c]["cpart"]
        for s, I in enumerate(sorted(BINS[c])):
            C[I * BLK : (I + 1) * BLK, I * BLK :] = cp[
                s * BLK : (s + 1) * BLK, I * BLK :
            ].astype(np.float32)
    return C
